# revision 9
# baseline (speedup 1.0000x reference)
"""CostVolume kernel for 8 TRN2 NeuronCores.

Strategy:
  out[b,d,h,w,o] = leaky_relu(mean_c(c1[b,d,h,w,c] * warpP[b,d+dz,h+dy,w+dx,c]))
  with o=(dz,dy,dx) in 5x5x5, warpP zero-padded by 2 in d,h,w.

  The channel contraction (C=64) runs on the TensorEngine as block-Gram
  matmuls: stationary = 32 c1 positions (8 d x 4 h x 1 w), moving = the
  480-column warp window (12 d' x 8 h' x 5 x'). Each PSUM Gram tile holds
  every (c1-pos, warp-pos) dot product; the useful "diagonal window" per
  partition is extracted on the host after the kernel ships the (leaky-
  relu'd) Gram tiles back. 4 blocks run concurrently via 32-column PE
  tile_position packing. Sharding: W axis split 8 ways (6 cols/core) with
  a +-2 halo on the padded warp volume.
"""

import sys

for p in ("/opt/trn_rl_repo", "/opt/trn_rl_repo/concourse"):
    if p not in sys.path:
        sys.path.insert(0, p)

import numpy as np
import ml_dtypes

BF16 = ml_dtypes.bfloat16

# problem constants (hardcoded per contract)
B, D, H, W, C = 2, 48, 48, 48, 64
S = 2                 # search range
M5 = 5                # offsets per axis
NCORES = 8
XL = W // NCORES      # 6 x-positions per core
RD, RH = 8, 4         # stationary block extent in d, h  (M = 32)
NDG = D // RD         # 6 d-blocks
NHG = H // (4 * RH)   # 3 h-quads (4 col-groups x RH)
NFS = B * NDG * NHG * XL   # 216 flight-sets per core
NCOLS = (RD + 4) * (RH + 4) * M5   # 12*8*5 = 480 columns per block
CHUNK = 24            # flight-sets per output DMA
NCHUNK = NFS // CHUNK  # 9
NSLOT = 2 * CHUNK     # staging slots (double buffer)

_cached = {}


def _build_bass():
    import concourse.bass as bass
    import concourse.mybir as mybir

    nc = bass.Bass()
    bf = mybir.dt.bfloat16
    f32 = mybir.dt.float32

    c1_ext = nc.declare_dram_parameter(
        "c1s", [64, B, NDG, NHG * 4, XL, RD * RH], bf, isOutput=False)
    wp_ext = nc.declare_dram_parameter("warps", [64, B, D + 4, H + 4, XL + 4], bf,
                                       isOutput=False)
    out_ext = nc.declare_dram_parameter("out", [128, NFS, NCOLS], bf, isOutput=True)

    with (
        nc.semaphore("in_sem") as in_sem,
        nc.semaphore("mm_sem") as mm_sem,
        nc.semaphore("act_sem") as act_sem,
        nc.semaphore("st_sem") as st_sem,
        nc.sbuf_tensor("c1sb", [64, B, NDG, NHG * 4, XL, RD * RH], bf) as c1sb,
        nc.sbuf_tensor("wpsb", [64, B, D + 4, H + 4, XL + 4], bf) as wpsb,
        nc.sbuf_tensor("stg", [128, NSLOT, NCOLS], bf) as stg,
        nc.psum_tensor("ps0", [128, NCOLS], f32) as ps0,
        nc.psum_tensor("ps1", [128, NCOLS], f32) as ps1,
        nc.psum_tensor("ps2", [128, NCOLS], f32) as ps2,
        nc.psum_tensor("ps3", [128, NCOLS], f32) as ps3,
    ):
        psum = [ps0, ps1, ps2, ps3]

        def fs_decode(fs):
            xl = fs % XL
            hq = (fs // XL) % NHG
            dg = (fs // (XL * NHG)) % NDG
            b = fs // (XL * NHG * NDG)
            return b, dg, hq, xl

        with nc.Block() as block:

            @block.sync
            def _(sync):
                sync.dma_start(out=c1sb[:], in_=c1_ext[:]).then_inc(in_sem, 16)
                sync.dma_start(out=wpsb[:], in_=wp_ext[:]).then_inc(in_sem, 16)
                # output store: one DMA per chunk of CHUNK flight-sets
                for ch in range(NCHUNK):
                    sync.wait_ge(act_sem, (ch + 1) * CHUNK)
                    slot = (ch % 2) * CHUNK
                    sync.dma_start(
                        out=out_ext[:, ch * CHUNK:(ch + 1) * CHUNK, :],
                        in_=stg[:, slot:slot + CHUNK, :],
                    ).then_inc(st_sem, 16)

            @block.tensor
            def _(tensor):
                tensor.wait_ge(in_sem, 32)
                for fs in range(NFS):
                    b, dg, hq, xl = fs_decode(fs)
                    if fs >= 4:
                        tensor.wait_ge(act_sem, fs - 3)
                    ps = psum[fs % 4]
                    for j in range(4):
                        h0 = (hq * 4 + j) * RH
                        d0 = dg * RD
                        lhsT = c1sb[:, b, dg, hq * 4 + j, xl, :]
                        rhs = wpsb[:, b, d0:d0 + RD + 4, h0:h0 + RH + 4,
                                   xl:xl + 5]
                        mm = tensor.matmul(
                            ps[32 * j:32 * (j + 1), :], lhsT, rhs,
                            start=True, stop=True,
                            tile_position=(0, 32 * j),
                        )
                    mm.then_inc(mm_sem)

            @block.scalar
            def _(scalar):
                for fs in range(NFS):
                    scalar.wait_ge(mm_sem, fs + 1)
                    ch = fs // CHUNK
                    if ch >= 2:
                        scalar.wait_ge(st_sem, (ch - 1) * 16)
                    slot = (ch % 2) * CHUNK + fs % CHUNK
                    scalar.activation(
                        stg[:, slot, :], psum[fs % 4][:, :],
                        mybir.ActivationFunctionType.Copy,
                    ).then_inc(act_sem)

    return nc


def _prep_inputs(c1, warp):
    """Host-side: scale, bf16-convert, channel-major transpose, shard, pad."""
    c1t = np.ascontiguousarray(
        np.transpose(c1.astype(np.float32) * (1.0 / C), (4, 0, 1, 2, 3))
    ).astype(BF16)                                     # [64, B, D, H, W]
    # reorder so each block's 32 stationary positions are contiguous:
    # [64, B, dg, hb, W, bd*RH+bh]
    c1r = c1t.reshape(64, B, NDG, RD, (NHG * 4), RH, W).transpose(
        0, 1, 2, 4, 6, 3, 5).reshape(64, B, NDG, NHG * 4, W, RD * RH)
    wp = np.pad(warp.astype(np.float32),
                ((0, 0), (S, S), (S, S), (S, S), (0, 0)))
    wpt = np.ascontiguousarray(
        np.transpose(wp, (4, 0, 1, 2, 3))
    ).astype(BF16)                                     # [64, B, D+4, H+4, W+4]
    in_maps = []
    for k in range(NCORES):
        in_maps.append({
            "c1s": np.ascontiguousarray(c1r[:, :, :, :, XL * k:XL * (k + 1), :]),
            "warps": np.ascontiguousarray(
                wpt[:, :, :, :, XL * k:XL * k + XL + 4]),
        })
    return in_maps


def _extract(grams):
    """Host-side: pull diagonal windows out of the Gram tiles.

    grams: [8, 128, NFS, NCOLS] float32 (already leaky-relu'd on device)
    returns [B, D, H, W, 125] float32
    """
    g = grams.reshape(NCORES, 4, RD, RH, B, NDG, NHG, XL, RD + 4, RH + 4, M5)
    O = np.empty((B, NDG, RD, NHG, 4, RH, NCORES, XL, M5, M5, M5),
                 dtype=np.float32)
    for bd in range(RD):
        for bh in range(RH):
            sel = g[:, :, bd, bh, :, :, :, :, bd:bd + M5, bh:bh + M5, :]
            # sel axes: (xg, j, b, dg, hq, xl, dz, dy, dx)
            O[:, :, bd, :, :, bh] = np.transpose(sel, (2, 3, 4, 1, 0, 5, 6, 7, 8))
    out = O.reshape(B, D, H, W, M5 ** 3)
    return np.where(out > 0, out, np.float32(0.1) * out)


def kernel(c1, warp):
    from concourse.bass_utils import run_bass_kernel_spmd

    if "nc" not in _cached:
        _cached["nc"] = _build_bass()
    nc = _cached["nc"]

    in_maps = _prep_inputs(np.asarray(c1), np.asarray(warp))
    res = run_bass_kernel_spmd(nc, in_maps, core_ids=list(range(NCORES)))
    grams = np.stack([np.asarray(res.results[i]["out"], dtype=np.float32)
                      for i in range(NCORES)])
    return _extract(grams)


# revision 15
# speedup vs baseline: 1.2332x; 1.2332x over previous
"""CostVolume kernel for 8 TRN2 NeuronCores.

Strategy:
  out[b,d,h,w,o] = leaky_relu(mean_c(c1[b,d,h,w,c] * warpP[b,d+dz,h+dy,w+dx,c]))
  with o=(dz,dy,dx) in 5x5x5, warpP zero-padded by 2 in d,h,w.

  The channel contraction (C=64) runs on the TensorEngine as block-Gram
  matmuls: stationary = 32 c1 positions (8 d x 4 h x 1 w), moving = the
  480-column warp window (12 d' x 8 h' x 5 x'). Each PSUM Gram tile holds
  every (c1-pos, warp-pos) dot product; the useful "diagonal window" per
  partition is extracted on the host after the kernel ships the (leaky-
  relu'd) Gram tiles back. 4 blocks run concurrently via 32-column PE
  tile_position packing. Sharding: W axis split 8 ways (6 cols/core) with
  a +-2 halo on the padded warp volume.
"""

import sys

for p in ("/opt/trn_rl_repo", "/opt/trn_rl_repo/concourse"):
    if p not in sys.path:
        sys.path.insert(0, p)

import numpy as np
import ml_dtypes

BF16 = ml_dtypes.bfloat16

# problem constants (hardcoded per contract)
B, D, H, W, C = 2, 48, 48, 48, 64
S = 2                 # search range
M5 = 5                # offsets per axis
NCORES = 8
XL = W // NCORES      # 6 x-positions per core
RD, RH = 8, 4         # stationary block extent in d, h  (M = 32)
NDG = D // RD         # 6 d-blocks
NHG = H // (4 * RH)   # 3 h-quads (4 col-groups x RH)
NFS = B * NDG * NHG * XL   # 216 flight-sets per core
NCOLS = (RD + 4) * (RH + 4) * M5   # 12*8*5 = 480 columns per block
CHUNK = 24            # flight-sets per output DMA
NCHUNK = NFS // CHUNK  # 9
NSLOT = 2 * CHUNK     # staging slots (double buffer)

_cached = {}


def _build_bass(nchunk=NCHUNK):
    import concourse.bass as bass
    import concourse.mybir as mybir

    nfs = nchunk * CHUNK
    nc = bass.Bass()
    bf = mybir.dt.bfloat16
    f32 = mybir.dt.float32

    c1_ext = nc.declare_dram_parameter(
        "c1s", [64, B, NDG, NHG * 4, XL, RD * RH], bf, isOutput=False)
    wp_ext = nc.declare_dram_parameter("warps", [64, B, D + 4, H + 4, XL + 4], bf,
                                       isOutput=False)
    out_ext = nc.declare_dram_parameter("out", [128, NFS, NCOLS], bf, isOutput=True)

    with (
        nc.semaphore("in_sem") as in_sem,
        nc.semaphore("mm_sem") as mm_sem,
        nc.semaphore("actA_sem") as actA_sem,
        nc.semaphore("actB_sem") as actB_sem,
        nc.semaphore("st_sem") as st_sem,
        nc.sbuf_tensor("c1sb", [64, B, NDG, NHG * 4, XL, RD * RH], bf) as c1sb,
        nc.sbuf_tensor("wpsb", [64, B, D + 4, H + 4, XL + 4], bf) as wpsb,
        nc.sbuf_tensor("stg", [128, NSLOT, NCOLS], bf) as stg,
        nc.psum_tensor("ps0", [128, 512], f32) as ps0,
        nc.psum_tensor("ps1", [128, 512], f32) as ps1,
        nc.psum_tensor("ps2", [128, 512], f32) as ps2,
        nc.psum_tensor("ps3", [128, 512], f32) as ps3,
    ):
        psum = [ps0, ps1, ps2, ps3]  # one full 2KB bank each

        def fs_decode(fs):
            xl = fs % XL
            hq = (fs // XL) % NHG
            dg = (fs // (XL * NHG)) % NDG
            b = fs // (XL * NHG * NDG)
            return b, dg, hq, xl

        with nc.Block() as block:

            @block.sync
            def _(sync):
                sync.dma_start(out=c1sb[:], in_=c1_ext[:]).then_inc(in_sem, 16)
                sync.dma_start(out=wpsb[:], in_=wp_ext[:]).then_inc(in_sem, 16)
                # output store: one DMA per chunk of CHUNK flight-sets
                for ch in range(nchunk):
                    half = (ch + 1) * CHUNK // 2
                    sync.wait_ge(actA_sem, half)
                    sync.wait_ge(actB_sem, half)
                    slot = (ch % 2) * CHUNK
                    sync.dma_start(
                        out=out_ext[:, ch * CHUNK:(ch + 1) * CHUNK, :],
                        in_=stg[:, slot:slot + CHUNK, :],
                    ).then_inc(st_sem, 16)

            @block.tensor
            def _(tensor):
                tensor.wait_ge(in_sem, 32)
                for fs in range(nfs):
                    b, dg, hq, xl = fs_decode(fs)
                    if fs >= 4:
                        # psum slot fs%4 was last used by fs-4 (same parity)
                        k = fs // 2
                        tensor.wait_ge(actA_sem if fs % 2 == 0 else actB_sem,
                                       k - 1)
                    ps = psum[fs % 4]
                    for j in range(4):
                        h0 = (hq * 4 + j) * RH
                        d0 = dg * RD
                        lhsT = c1sb[:, b, dg, hq * 4 + j, xl, :]
                        rhs = wpsb[:, b, d0:d0 + RD + 4, h0:h0 + RH + 4,
                                   xl:xl + 5]
                        mm = tensor.matmul(
                            ps[32 * j:32 * (j + 1), 0:NCOLS], lhsT, rhs,
                            start=True, stop=True,
                            tile_position=(0, 32 * j),
                        )
                    mm.then_inc(mm_sem)

            @block.scalar
            def _(scalar):
                for fs in range(0, nfs, 2):   # even flight-sets
                    scalar.wait_ge(mm_sem, fs + 1)
                    ch = fs // CHUNK
                    if ch >= 2:
                        scalar.wait_ge(st_sem, (ch - 1) * 16)
                    slot = (ch % 2) * CHUNK + fs % CHUNK
                    scalar.activation(
                        stg[:, slot, :], psum[fs % 4][:, 0:NCOLS],
                        mybir.ActivationFunctionType.Copy,
                    ).then_inc(actA_sem)

            @block.vector
            def _(vector):
                for fs in range(1, nfs, 2):   # odd flight-sets
                    vector.wait_ge(mm_sem, fs + 1)
                    ch = fs // CHUNK
                    if ch >= 2:
                        vector.wait_ge(st_sem, (ch - 1) * 16)
                    slot = (ch % 2) * CHUNK + fs % CHUNK
                    vector.tensor_copy(stg[:, slot, :],
                                       psum[fs % 4][:, 0:NCOLS]).then_inc(actB_sem)

    return nc


def _prep_inputs(c1, warp):
    """Host-side: scale, bf16-convert, channel-major transpose, shard, pad."""
    c1t = np.ascontiguousarray(
        np.transpose(c1.astype(np.float32) * (1.0 / C), (4, 0, 1, 2, 3))
    ).astype(BF16)                                     # [64, B, D, H, W]
    # reorder so each block's 32 stationary positions are contiguous:
    # [64, B, dg, hb, W, bd*RH+bh]
    c1r = c1t.reshape(64, B, NDG, RD, (NHG * 4), RH, W).transpose(
        0, 1, 2, 4, 6, 3, 5).reshape(64, B, NDG, NHG * 4, W, RD * RH)
    wp = np.pad(warp.astype(np.float32),
                ((0, 0), (S, S), (S, S), (S, S), (0, 0)))
    wpt = np.ascontiguousarray(
        np.transpose(wp, (4, 0, 1, 2, 3))
    ).astype(BF16)                                     # [64, B, D+4, H+4, W+4]
    in_maps = []
    for k in range(NCORES):
        in_maps.append({
            "c1s": np.ascontiguousarray(c1r[:, :, :, :, XL * k:XL * (k + 1), :]),
            "warps": np.ascontiguousarray(
                wpt[:, :, :, :, XL * k:XL * k + XL + 4]),
        })
    return in_maps


def _extract(grams):
    """Host-side: pull diagonal windows out of the Gram tiles.

    grams: [8, 128, NFS, NCOLS] float32 (already leaky-relu'd on device)
    returns [B, D, H, W, 125] float32
    """
    g = grams.reshape(NCORES, 4, RD, RH, B, NDG, NHG, XL, RD + 4, RH + 4, M5)
    O = np.empty((B, NDG, RD, NHG, 4, RH, NCORES, XL, M5, M5, M5),
                 dtype=np.float32)
    for bd in range(RD):
        for bh in range(RH):
            sel = g[:, :, bd, bh, :, :, :, :, bd:bd + M5, bh:bh + M5, :]
            # sel axes: (xg, j, b, dg, hq, xl, dz, dy, dx)
            O[:, :, bd, :, :, bh] = np.transpose(sel, (2, 3, 4, 1, 0, 5, 6, 7, 8))
    out = O.reshape(B, D, H, W, M5 ** 3)
    return np.where(out > 0, out, np.float32(0.1) * out)


def kernel(c1, warp):
    from concourse.bass_utils import run_bass_kernel_spmd

    if "nc" not in _cached:
        _cached["nc"] = _build_bass()
    nc = _cached["nc"]

    in_maps = _prep_inputs(np.asarray(c1), np.asarray(warp))
    res = run_bass_kernel_spmd(nc, in_maps, core_ids=list(range(NCORES)))
    grams = np.stack([np.asarray(res.results[i]["out"], dtype=np.float32)
                      for i in range(NCORES)])
    return _extract(grams)
